# revision 1
# baseline (speedup 1.0000x reference)
"""Trainium2 Bass kernel for nn_ConstGCN.

Math note: in the reference, the attention score s[b,i] is constant along
the softmax axis j, and softmax is shift-invariant, so
p = softmax(s + mask) = softmax(mask) and p.sum(axis=2) == 1 (to ~1e-6 in
f32).  The output therefore collapses to

    out = relu(text + mean_k(emb_table[const_labels[...,k]]) @ fc_W.T + fc_b)

which depends on neither const_mat nor attn_W/attn_b.  The embedding + fc
further fuse into a single gather table M2 = (emb_table @ fc_W.T)/8, so

    out[b,l,:] = relu(text[b,l,:] + sum_k M2[labels[b,l,k], :] + fc_b)

On device (per core, data-parallel over batch: 2 of 16 batches = 4096
positions):
  - DVE builds one-hot counts over the 100 label classes with an fp16
    is_equal against a replicated iota constant, then reduces over K=8
  - PE transposes counts to [class, position] via identity matmul; class
    row 100 (count 1, M2 row 100 = fc_b) is set by a GPSIMD memset,
    which folds the bias into the matmul
  - PE matmul counts.T @ M2 (fp16 in, f32 accumulate) -> PSUM
  - ACT copies PSUM -> SBUF, a DMA with accum_op=add streams text from
    HBM directly onto it, GPSIMD applies relu in place, DMA streams out
const_mat (256 MiB) is never read.
"""

import numpy as np
import ml_dtypes

B, L, D = 16, 2048, 256
CN, K = 100, 8
NCLS = 128         # 100 label classes + bias class 100 (M2 row 100 = fc_b,
                   # rows 101..127 zero; count rows 100..127 are constant 1)
NCORES = 8
POS = (B // NCORES) * L          # 4096 positions per core
CHUNK = 512                      # positions per chunk
NCHUNK = POS // CHUNK            # 8
Q = CHUNK // 128                 # 4 position-groups of 128 per chunk

_compiled = None


def _build():
    import concourse.bacc as bacc
    import concourse.mybir as mybir
    from concourse.tile import TileContext

    f32 = mybir.dt.float32
    fp16 = mybir.dt.float16
    bf16 = mybir.dt.bfloat16

    nc = bacc.Bacc("TRN2", target_bir_lowering=False)

    text_d = nc.dram_tensor("text", [POS, D], f32, kind="ExternalInput")
    lab_d = nc.dram_tensor("labels", [NCHUNK, 128, Q * K], bf16,
                           kind="ExternalInput")
    m2_d = nc.dram_tensor("m2", [NCLS, D], fp16, kind="ExternalInput")
    out_d = nc.dram_tensor("out", [POS, D], f32, kind="ExternalOutput")

    # constants embedded in the NEFF
    iota_np = np.repeat(np.arange(CN, dtype=np.float32), K)  # [CN*K]
    iota_np = np.broadcast_to(iota_np, (128, CN * K)).astype(ml_dtypes.bfloat16)
    iota_d = nc.inline_tensor(np.ascontiguousarray(iota_np), name="iota")
    ident_d = nc.inline_tensor(np.eye(128, dtype=ml_dtypes.bfloat16), name="ident")

    # position index within a chunk: partition p, group q  <->  p*Q + q
    text_v = text_d.rearrange("(n p q) d -> n p (q d)", p=128, q=Q)
    out_v = out_d.rearrange("(n p q) d -> n p (q d)", p=128, q=Q)

    with TileContext(nc) as tc:
        with (
            tc.tile_pool(name="const", bufs=1) as cpool,
            tc.tile_pool(name="work", bufs=2) as wpool,
            tc.tile_pool(name="io", bufs=3) as iopool,
            tc.tile_pool(name="ps_t", bufs=4, space="PSUM") as pst,
            tc.tile_pool(name="ps_a", bufs=4, space="PSUM") as psa,
        ):
            iota_sb = cpool.tile([128, CN * K], bf16)
            nc.sync.dma_start(out=iota_sb[:, :], in_=iota_d[:, :])
            ident_sb = cpool.tile([128, 128], bf16)
            nc.sync.dma_start(out=ident_sb[:, :], in_=ident_d[:, :])
            m2_sb = cpool.tile([NCLS, D], fp16)
            nc.sync.dma_start(out=m2_sb[:, :], in_=m2_d[:, :])

            # persistent counts.T tiles (one per q-pair); rows 96..127 are
            # set to one exactly once — the per-pair copy only overwrites
            # 0..99, so row 100 stays 1 and feeds the fc_b row of m2
            ct_tiles = []
            for i in range(Q // 2):
                ct = cpool.tile([NCLS, 256], fp16, tag=f"ct{i}")
                nc.gpsimd.memset(ct[96:128, :], 1.0)
                ct_tiles.append(ct)

            for n in range(NCHUNK):
                lab = wpool.tile([128, Q * K], bf16, tag="lab")
                nc.sync.dma_start(out=lab[:, :], in_=lab_d[n, :, :])

                eq = wpool.tile([128, Q * CN * K], bf16, tag="eq")
                eq3 = eq.rearrange("p (q c k) -> p q c k", c=CN, k=K)
                nc.vector.tensor_tensor(
                    out=eq3,
                    in0=lab.rearrange("p (q k) -> p q k", k=K)[:, :, None, :]
                        .broadcast_to([128, Q, CN, K]),
                    in1=iota_sb.rearrange("p (c k) -> p c k", k=K)[:, None, :, :]
                        .broadcast_to([128, Q, CN, K]),
                    op=mybir.AluOpType.is_equal,
                )

                # sum over k via a TT-add tree: tensor_tensor has a 2x bf16
                # mode, tensor_reduce does not (measured 1x)
                s1 = wpool.tile([128, Q * CN * 4], bf16, tag="s1")
                s13 = s1.rearrange("p (q c k) -> p q c k", c=CN, k=4)
                nc.vector.tensor_add(out=s13, in0=eq3[:, :, :, 0:4],
                                     in1=eq3[:, :, :, 4:8])
                s2 = wpool.tile([128, Q * CN * 2], bf16, tag="s2")
                s23 = s2.rearrange("p (q c k) -> p q c k", c=CN, k=2)
                nc.vector.tensor_add(out=s23, in0=s13[:, :, :, 0:2],
                                     in1=s13[:, :, :, 2:4])
                counts = wpool.tile([128, Q * CN], bf16, tag="counts")
                nc.vector.tensor_add(
                    out=counts.rearrange("p (q c) -> p q c", c=CN),
                    in0=s23[:, :, :, 0],
                    in1=s23[:, :, :, 1],
                )

                res = iopool.tile([128, Q * D], f32, tag="res")
                for h in range(Q // 2):
                    ctp = pst.tile([CN, 256], bf16, tag="ctp")
                    for j in range(2):
                        q = 2 * h + j
                        nc.tensor.transpose(
                            out=ctp[:, j * 128:(j + 1) * 128],
                            in_=counts[:, q * CN:(q + 1) * CN],
                            identity=ident_sb[:, :],
                        )
                    ct = ct_tiles[h]
                    nc.scalar.copy(out=ct[:CN, :], in_=ctp[:, :])

                    acc = psa.tile([128, 2 * D], f32, tag="acc")
                    for j in range(2):
                        nc.tensor.matmul(
                            acc[:, j * D:(j + 1) * D],
                            lhsT=ct[:, j * 128:(j + 1) * 128],
                            rhs=m2_sb[:, :],
                            start=True, stop=True,
                        )
                    nc.scalar.copy(out=res[:, h * 2 * D:(h + 1) * 2 * D],
                                   in_=acc[:, :])

                # res += text  (streamed straight from HBM by the DMA engines)
                nc.gpsimd.dma_start(out=res[:, :], in_=text_v[n, :, :],
                                    accum_op=mybir.AluOpType.add)
                nc.vector.tensor_scalar_max(out=res[:, :], in0=res[:, :],
                                            scalar1=0.0)
                nc.sync.dma_start(out=out_v[n, :, :], in_=res[:, :])

    nc.finalize()
    return nc


def _get_compiled():
    global _compiled
    if _compiled is None:
        _compiled = _build()
    return _compiled


def _prep_core_inputs(text, labels_fp16, m2):
    """text: [POS, D] f32, labels_fp16: [POS, K] fp16 -> in_map."""
    lab = labels_fp16.reshape(NCHUNK, 128, Q, K)  # (n, p, q, k): pos = n*CHUNK + p*Q + q
    lab = np.ascontiguousarray(lab.reshape(NCHUNK, 128, Q * K))
    return {
        "text": np.ascontiguousarray(text),
        "labels": lab,
        "m2": m2,
    }


def kernel(text, const_mat, const_labels, emb_table, attn_W, attn_b,
           fc_W, fc_b):
    from concourse.bass_utils import run_bass_kernel_spmd

    text = np.asarray(text, dtype=np.float32)
    const_labels = np.asarray(const_labels)
    emb_table = np.asarray(emb_table, dtype=np.float32)
    fc_W = np.asarray(fc_W, dtype=np.float32)
    fc_b = np.asarray(fc_b, dtype=np.float32)

    # fused gather table: row c (c<CN) = (emb_table @ fc_W.T)[c]/8,
    # row 100 = fc_b (count rows 100..127 are constant 1; 101..127 are 0)
    m2 = np.zeros((NCLS, D), dtype=np.float64)
    m2[:CN] = emb_table.astype(np.float64) @ fc_W.T.astype(np.float64) * 0.125
    m2[CN] = fc_b
    m2 = m2.astype(np.float16)

    lab_bf16 = const_labels.reshape(B * L, K).astype(ml_dtypes.bfloat16)
    text_flat = text.reshape(B * L, D)

    nc = _get_compiled()
    in_maps = []
    for c in range(NCORES):
        sl = slice(c * POS, (c + 1) * POS)
        in_maps.append(_prep_core_inputs(text_flat[sl], lab_bf16[sl], m2))

    r = run_bass_kernel_spmd(nc, in_maps, core_ids=list(range(NCORES)))
    out = np.concatenate([r.results[c]["out"] for c in range(NCORES)], axis=0)
    return out.reshape(B, L, D)



# revision 7
# speedup vs baseline: 2.0225x; 2.0225x over previous
"""Trainium2 Bass kernel for nn_ConstGCN.

Math note: in the reference, the attention score s[b,i] is constant along
the softmax axis j, and softmax is shift-invariant, so
p = softmax(s + mask) = softmax(mask) and p.sum(axis=2) == 1 (to ~1e-6 in
f32).  The output therefore collapses to

    out = relu(text + mean_k(emb_table[const_labels[...,k]]) @ fc_W.T + fc_b)

which depends on neither const_mat nor attn_W/attn_b.  The embedding + fc
fuse into a single table M2 = (emb_table @ fc_W.T)/8, so

    out[b,l,:] = relu(text[b,l,:] + sum_k M2[labels[b,l,k], :] + fc_b)

Input marshalling on host: the integer labels [pos, 8] are re-encoded as
per-position class-count vectors (np.bincount; counts in 0..8 are exact in
fp8e4m3), shipped transposed as [class, pos] so the device consumes them
directly as the matmul stationary.  Class row 100 is constant 1 and M2 row
100 = fc_b, folding the bias into the matmul; rows 101..127 are zero.
text is shipped as bf16, the output as fp16 (tolerance is 2e-2; these add
~4e-3).

On device (per core, data-parallel over batch: 2 of 16 batches = 4096
positions, in 8 chunks of 512):
  - PE: one identity matmul streams the text chunk into PSUM (start=True),
    then four fp8xbf16 matmuls accumulate counts.T @ M2 on top
  - relu + cast f32->fp16 evicts PSUM, split between ACT and DVE
  - HWDGE DMAs: text in on the sync ring, out on the scalar ring,
    counts via one SWDGE transfer up front
const_mat (256 MiB) is never read.
"""

import numpy as np
import ml_dtypes

B, L, D = 16, 2048, 256
CN, K = 100, 8
NCLS = 128         # 100 label classes + bias class 100 (M2 row 100 = fc_b)
NCORES = 8
POS = (B // NCORES) * L          # 4096 positions per core
CHUNK = 512                      # positions per chunk
NCHUNK = POS // CHUNK            # 8
Q = CHUNK // 128                 # 4 position-groups of 128 per chunk

_compiled = None


def _build():
    import concourse.bacc as bacc
    import concourse.mybir as mybir
    from concourse.tile import TileContext

    f32 = mybir.dt.float32
    fp16 = mybir.dt.float16
    bf16 = mybir.dt.bfloat16
    fp8 = mybir.dt.float8e4

    nc = bacc.Bacc("TRN2", target_bir_lowering=False)

    text_d = nc.dram_tensor("text", [NCHUNK, 128, Q * D], bf16,
                            kind="ExternalInput")
    ct_d = nc.dram_tensor("ct", [128, NCHUNK * Q * 128], fp8,
                          kind="ExternalInput")
    m2_d = nc.dram_tensor("m2", [NCLS, D], bf16, kind="ExternalInput")
    out_d = nc.dram_tensor("out", [NCHUNK, 128, Q * D], fp16,
                           kind="ExternalOutput")

    ident_d = nc.inline_tensor(np.eye(128, dtype=ml_dtypes.bfloat16),
                               name="ident")

    with TileContext(nc) as tc:
        with (
            tc.tile_pool(name="const", bufs=1) as cpool,
            tc.tile_pool(name="in", bufs=3) as ipool,
            tc.tile_pool(name="res", bufs=3) as rpool,
            tc.tile_pool(name="ps", bufs=3, space="PSUM") as pst,
        ):
            ident_sb = cpool.tile([128, 128], bf16)
            nc.sync.dma_start(out=ident_sb[:, :], in_=ident_d[:, :])
            m2_sb = cpool.tile([NCLS, D], bf16)
            nc.sync.dma_start(out=m2_sb[:, :], in_=m2_d[:, :])
            # all count-vectors up front in one SWDGE transfer (512 KiB)
            ct_sb = cpool.tile([128, NCHUNK * Q * 128], fp8)
            ct_v = ct_sb.rearrange("p (n x) -> p n x", n=NCHUNK)
            nc.gpsimd.dma_start(out=ct_sb[:, :], in_=ct_d[:, :])

            for n in range(NCHUNK):
                text_t = ipool.tile([128, Q * D], bf16, tag="text")
                nc.sync.dma_start(out=text_t[:, :], in_=text_d[n, :, :])

                acc = pst.tile([128, Q * D], f32, tag="acc")
                # matmul free size caps at 512 (one PSUM bank)
                for h in range(2):
                    nc.tensor.matmul(acc[:, h * 512:(h + 1) * 512],
                                     lhsT=ident_sb[:, :],
                                     rhs=text_t[:, h * 512:(h + 1) * 512],
                                     start=True, stop=False)
                for q in range(Q):
                    nc.tensor.matmul(
                        acc[:, q * D:(q + 1) * D],
                        lhsT=ct_v[:, n, q * 128:(q + 1) * 128],
                        rhs=m2_sb[:, :],
                        start=False, stop=True,
                    )

                res = rpool.tile([128, Q * D], fp16, tag="res")
                h = Q * D // 2
                nc.scalar.activation(res[:, :h], acc[:, :h],
                                     mybir.ActivationFunctionType.Relu)
                nc.vector.tensor_scalar_max(out=res[:, h:], in0=acc[:, h:],
                                            scalar1=0.0)
                nc.scalar.dma_start(out=out_d[n, :, :], in_=res[:, :])

    nc.finalize()
    return nc


def _get_compiled():
    global _compiled
    if _compiled is None:
        _compiled = _build()
    return _compiled


def _host_prep(text, const_labels, emb_table, fc_W, fc_b):
    """Marshal full inputs -> per-core in_maps."""
    # fused gather table: row c (c<CN) = (emb_table @ fc_W.T)[c]/8,
    # row 100 = fc_b (count row 100 is constant 1), rows 101..127 zero
    m2 = np.zeros((NCLS, D), dtype=np.float64)
    m2[:CN] = emb_table.astype(np.float64) @ fc_W.T.astype(np.float64) * 0.125
    m2[CN] = fc_b
    m2 = m2.astype(ml_dtypes.bfloat16)

    # label -> count-vector encoding (counts 0..8, exact in fp8e4m3)
    lab = np.ascontiguousarray(const_labels.reshape(B * L, K)).astype(np.int64)
    ids = (np.arange(B * L, dtype=np.int64) * CN)[:, None] + lab
    counts = np.bincount(ids.ravel(), minlength=B * L * CN).reshape(B * L, CN)
    # layout per core: [class, n, q*128 + p] with pos = n*512 + p*4 + q
    cc = counts.reshape(NCORES, NCHUNK, 128, Q, CN)
    ct = np.zeros((NCORES, NCLS, NCHUNK, Q, 128), dtype=np.float32)
    ct[:, :CN] = cc.transpose(0, 4, 1, 3, 2)
    ct[:, CN] = 1.0
    ct = ct.reshape(NCORES, NCLS, NCHUNK * Q * 128).astype(
        ml_dtypes.float8_e4m3fn)

    text16 = np.ascontiguousarray(text.reshape(B * L, D)).astype(
        ml_dtypes.bfloat16)
    text16 = text16.reshape(NCORES, NCHUNK, 128, Q * D)

    in_maps = []
    for c in range(NCORES):
        in_maps.append({
            "text": np.ascontiguousarray(text16[c]),
            "ct": np.ascontiguousarray(ct[c]),
            "m2": m2,
        })
    return in_maps


def kernel(text, const_mat, const_labels, emb_table, attn_W, attn_b,
           fc_W, fc_b):
    from concourse.bass_utils import run_bass_kernel_spmd

    text = np.asarray(text, dtype=np.float32)
    const_labels = np.asarray(const_labels)
    emb_table = np.asarray(emb_table, dtype=np.float32)
    fc_W = np.asarray(fc_W, dtype=np.float32)
    fc_b = np.asarray(fc_b, dtype=np.float32)

    in_maps = _host_prep(text, const_labels, emb_table, fc_W, fc_b)
    nc = _get_compiled()
    r = run_bass_kernel_spmd(nc, in_maps, core_ids=list(range(NCORES)))
    out = np.stack([r.results[c]["out"] for c in range(NCORES)], axis=0)
    # [core, n, p, q*d]: position = ((core*8 + n)*128 + p)*4 + q, so a
    # plain reshape restores position order
    return out.astype(np.float32).reshape(B, L, D)
